# revision 1
# baseline (speedup 1.0000x reference)
"""Multi-head attention block (QKV projections + softmax attention + output
projection) for TRN2, distributed over 8 NeuronCores; fp16 data path with
fp32 PSUM accumulation, early-exp schedule.

Sharding: core c handles batch b = c // 2 and head group g = c % 2 (8 of the
16 heads).  Host sums the two partial outputs per batch and adds bo.

Schedule (the change vs v1): kt+wk then qt+wq stream first on the SP DMA
queue; K then Q are projected; TWO score blocks are emitted immediately
after, so the scalar engine's exp stream (the critical serial resource,
~65us of Exp) starts right after the Q projection instead of after the V
projection; a third score block interleaves mid-V-projection.  The drain
consumes P@V tasks FIFO so the early blocks' P^T tiles are freed promptly.
Output projection matmuls run jc-outer so each stationary ot slice is
loaded once for both 512-column moving halves.
"""

from collections import deque
from contextlib import ExitStack

import numpy as np

import concourse.bass as bass
import concourse.tile as tile
from concourse import bacc, mybir
from concourse.bass_utils import run_bass_kernel_spmd

NUM_HEADS = 16
B = 4
N = 1024          # sequence length (n_q == n_k)
D = 1024          # model dim
OUT = 1024        # output dim
HD = 64           # head dim
NH = 8            # heads per core (16 heads / 2 groups)
J = NH * HD       # per-core all-head dim = 512
P = 128           # SBUF partitions
ND = D // P       # 8 contraction chunks over D
NKT = N // P      # 8 key-token chunks
NJ = J // P       # 4 chunks over j
QW = 512          # matmul moving width / PSUM bank width (fp32)
NQC = N // QW     # 2 query-column halves

F32 = mybir.dt.float32

DT = mybir.dt.float16
NPDT = np.float16
MASK_BIAS = -30000.0


def _build(use_mask: bool, use_bias: bool, reps: int = 1):
    nc = bacc.Bacc(None, target_bir_lowering=False)

    def din(nm, shape, dt=DT):
        return nc.declare_dram_parameter(nm, shape, dt, isOutput=False)

    qt_d, kt_d, vt_d = din("qt", [D, N]), din("kt", [D, N]), din("vt", [D, N])
    wq_d, wk_d, wv_d = din("wq", [D, J]), din("wk", [D, J]), din("wv", [D, J])
    wo_d = din("wo", [J, OUT])
    if use_bias:
        bqs_d, bks_d, bvs_d = (
            din("bqs", [1, J]), din("bks", [1, J]), din("bvs", [1, J])
        )
    mb_d = din("mb", [N, N], DT) if use_mask else None
    OUT_DT = DT
    out_d = nc.declare_dram_parameter("out", [N, OUT], OUT_DT, isOutput=True)

    AF = mybir.ActivationFunctionType
    big_bufs = 14

    with tile.TileContext(nc) as tc:
        with ExitStack() as ctx:
            # streaming slots for input chunks.
            big = ctx.enter_context(tc.tile_pool(name="big", bufs=big_bufs))
            # exp'd P^T tiles (256 KB fp16): own ring, sized for three
            # score blocks in flight plus margin.
            ppt = ctx.enter_context(tc.tile_pool(name="ppt", bufs=32))
            pacts = ctx.enter_context(tc.tile_pool(name="acts", bufs=1))
            pwo = ctx.enter_context(tc.tile_pool(name="pwo", bufs=1))
            pout = ctx.enter_context(tc.tile_pool(name="outc", bufs=2))
            psml = ctx.enter_context(tc.tile_pool(name="small", bufs=1))
            # psum: 3 x 2-bank score/proj tiles + 2 x 1-bank P@V tiles.
            psum = ctx.enter_context(tc.tile_pool(name="ps", bufs=3, space="PSUM"))
            if use_mask:
                pmask = ctx.enter_context(tc.tile_pool(name="pmask", bufs=1))

            # Small persistent tiles.
            onesf = psml.tile([1, QW], F32, name="onesf", tag="onesf")
            nc.vector.memset(onesf[:], 1.0)
            if use_bias:
                ones_row = psml.tile([1, QW], DT, name="ones_row",
                                     tag="ones_row")
                nc.vector.tensor_copy(ones_row[:], onesf[:])
                bqs_t = psml.tile([1, J], DT, name="bqs_t", tag="bqs")
                bks_t = psml.tile([1, J], DT, name="bks_t", tag="bks")
                bvs_t = psml.tile([1, J], DT, name="bvs_t", tag="bvs")
                nc.sync.dma_start(bqs_t[:], bqs_d[:])
                nc.sync.dma_start(bks_t[:], bks_d[:])
                nc.sync.dma_start(bvs_t[:], bvs_d[:])
            vof = psml.tile([P, NKT, NH, 1], DT, name="vof", tag="vof")
            nc.vector.memset(vof[:], 1.0)

            # Warm the ACT exp table while DMAs run.
            warm = psml.tile([1, QW], F32, name="warm", tag="rc", bufs=2)
            nc.scalar.activation(warm[:], onesf[:], AF.Exp)

            if use_mask:
                mb_t = pmask.tile([P, NKT, N], DT, name="mb_t", tag="mask")
                nc.sync.dma_start(
                    mb_t[:], mb_d[:].rearrange("(a p) n -> p a n", p=P)
                )

            def _emit_rep():
                # --- input chunks, first-use order, single SP queue.
                # w: 2 tiles x [P, 4, J]; x: 4 tiles x [P, 2, N].
                def alloc_wx(nm):
                    w = [big.tile([P, 4, J], DT, name=f"w{nm}{i}", tag="big")
                         for i in range(2)]
                    x = [big.tile([P, 2, N], DT, name=f"x{nm}{i}", tag="big")
                         for i in range(4)]
                    return w, x

                def dma_wx(wd, xd, w, x, order):
                    for lst, i in order:
                        if lst is w:
                            nc.sync.dma_start(
                                w[i][:],
                                wd[i * (4 * P):(i + 1) * (4 * P), :].rearrange(
                                    "(a p) j -> p a j", p=P
                                ),
                            )
                        else:
                            nc.sync.dma_start(
                                x[i][:],
                                xd[i * (2 * P):(i + 1) * (2 * P), :].rearrange(
                                    "(a p) n -> p a n", p=P
                                ),
                            )

                def load_wx(wd, xd, nm):
                    w, x = alloc_wx(nm)
                    dma_wx(wd, xd, w, x,
                           [(w, 0), (x, 0), (x, 1), (w, 1), (x, 2), (x, 3)])
                    return w, x

                qpt = pacts.tile([P, NJ, N], DT, name="qpt", tag="qpt")
                kpt = pacts.tile([P, NJ, N], DT, name="kpt", tag="kpt")
                vext = pacts.tile([P, NKT, NH, HD + 1], DT, name="vext",
                                  tag="vext")
                ot = pacts.tile([P, NJ, N], DT, name="ot", tag="ot")

                # --- Q/K projections over PSUM groups of 3 + 1 c-chunks.
                def qk_proj_pass(nm, w, x, bias_t, dst, cs, after_pass=None):
                    groups = {
                        c: psum.tile([P, NQC, QW], F32, name="psp",
                                     tag="ps")
                        for c in cs
                    }
                    if use_bias:
                        for c in cs:
                            for qc in range(NQC):
                                nc.tensor.matmul(
                                    groups[c][:, qc, :],
                                    bias_t[0:1, c * P:(c + 1) * P],
                                    ones_row[:], start=True, stop=False,
                                    skip_group_check=True,
                                )
                    for dc in range(ND):
                        for c in cs:
                            for qc in range(NQC):
                                nc.tensor.matmul(
                                    groups[c][:, qc, :],
                                    w[dc // 4][:, dc % 4,
                                               c * P:(c + 1) * P],
                                    x[dc // 2][:, dc % 2,
                                               qc * QW:(qc + 1) * QW],
                                    start=(dc == 0 and not use_bias),
                                    stop=(dc == ND - 1),
                                    skip_group_check=True,
                                )
                    for c in cs:
                        nc.vector.tensor_copy(
                            dst[:, c, :],
                            groups[c][:].rearrange("p a q -> p (a q)"),
                        )

                def qk_proj(nm, w, x, bias_t, dst):
                    qk_proj_pass(nm, w, x, bias_t, dst, range(3))
                    qk_proj_pass(nm, w, x, bias_t, dst, range(3, NJ))

                # kt/qt chunks interleaved on the queue so the Q projection
                # trails the K projection closely.
                kw, kx = alloc_wx("k")
                qw_, qx = alloc_wx("q")
                dma_wx(wk_d, kt_d, kw, kx, [(kw, 0), (kx, 0), (kx, 1)])
                dma_wx(wq_d, qt_d, qw_, qx, [(qw_, 0), (qx, 0), (qx, 1)])
                dma_wx(wk_d, kt_d, kw, kx, [(kw, 1), (kx, 2), (kx, 3)])
                dma_wx(wq_d, qt_d, qw_, qx, [(qw_, 1), (qx, 2), (qx, 3)])
                qk_proj("k", kw, kx, bks_t if use_bias else None, kpt)

                # --- Attention blocks; head pairs row-tiled in the PE.
                def s_block(pr, qc, pts):
                    for h in (2 * pr, 2 * pr + 1):
                        pts[(h, qc)] = [
                            ppt.tile([P, 2, QW], DT, name="pt", tag="pt")
                            for _ in range(NKT // 2)
                        ]
                    for kcp in range(NKT // 2):
                        pss = {}
                        for h in (2 * pr, 2 * pr + 1):
                            pss[h] = psum.tile([P, 2, QW], F32, name="pss",
                                               tag="ps")
                        for i in range(2):
                            kc = 2 * kcp + i
                            for h in (2 * pr, 2 * pr + 1):
                                off = HD * (h & 1)
                                nc.tensor.matmul(
                                    pss[h][:, i, :],
                                    kpt[off:off + HD, pr, kc * P:(kc + 1) * P],
                                    qpt[off:off + HD, pr, qc * QW:(qc + 1) * QW],
                                    start=True, stop=True,
                                    skip_group_check=True,
                                )
                        for h in (2 * pr, 2 * pr + 1):
                            if use_mask:
                                nc.vector.tensor_add(
                                    pss[h][:],
                                    pss[h][:],
                                    mb_t[:, 2 * kcp:2 * kcp + 2,
                                         qc * QW:(qc + 1) * QW],
                                )
                            nc.scalar.activation(
                                pts[(h, qc)][kcp][:], pss[h][:], AF.Exp,
                            )

                def pv_norm(po_, hv, qcv):
                    rc = psml.tile([1, QW], F32, name="rc", tag="rc", bufs=2)
                    nc.vector.reciprocal(rc[:], po_[HD:HD + 1, :])
                    rb = psml.tile([HD, QW], F32, name="rb", tag="rb", bufs=2)
                    nc.gpsimd.partition_broadcast(rb[:], rc[:])
                    off = HD * (hv & 1)
                    nc.vector.tensor_mul(
                        ot[off:off + HD, hv // 2, qcv * QW:(qcv + 1) * QW],
                        po_[0:HD, :], rb[:],
                    )

                def pv_block(h, qc, pt):
                    po = psum.tile([P, QW], F32, name="po", tag="po",
                                   bufs=2)
                    for kc in range(NKT):
                        nc.tensor.matmul(
                            po[0:HD + 1, :], vext[:, kc, h, :],
                            pt[kc // 2][:, kc % 2, :],
                            start=(kc == 0), stop=(kc == NKT - 1),
                        )
                    pv_norm(po, h, qc)

                def fused_block(s_task, pv_task, pts):
                    """Score block for s_task with the P@V pair for pv_task
                    interleaved at kc-pair steps."""
                    pr, qc = s_task
                    prv, qcv = pv_task
                    pta = pts.pop((2 * prv, qcv))
                    ptb = pts.pop((2 * prv + 1, qcv))
                    poa = psum.tile([P, QW], F32, name="po", tag="po",
                                    bufs=2)
                    pob = psum.tile([P, QW], F32, name="po", tag="po",
                                    bufs=2)
                    for h in (2 * pr, 2 * pr + 1):
                        pts[(h, qc)] = [
                            ppt.tile([P, 2, QW], DT, name="pt", tag="pt")
                            for _ in range(NKT // 2)
                        ]
                    for kcp in range(NKT // 2):
                        pss = {}
                        for h in (2 * pr, 2 * pr + 1):
                            pss[h] = psum.tile([P, 2, QW], F32, name="pss",
                                               tag="ps")
                        for i in range(2):
                            kc = 2 * kcp + i
                            for h in (2 * pr, 2 * pr + 1):
                                off = HD * (h & 1)
                                nc.tensor.matmul(
                                    pss[h][:, i, :],
                                    kpt[off:off + HD, pr, kc * P:(kc + 1) * P],
                                    qpt[off:off + HD, pr, qc * QW:(qc + 1) * QW],
                                    start=True, stop=True,
                                    skip_group_check=True,
                                )
                        for h in (2 * pr, 2 * pr + 1):
                            if use_mask:
                                nc.vector.tensor_add(
                                    pss[h][:],
                                    pss[h][:],
                                    mb_t[:, 2 * kcp:2 * kcp + 2,
                                         qc * QW:(qc + 1) * QW],
                                )
                            nc.scalar.activation(
                                pts[(h, qc)][kcp][:], pss[h][:], AF.Exp,
                            )
                        for po_, pt_, hv in ((poa, pta, 2 * prv),
                                             (pob, ptb, 2 * prv + 1)):
                            for i in range(2):
                                kc = 2 * kcp + i
                                nc.tensor.matmul(
                                    po_[0:HD + 1, :], vext[:, kc, hv, :],
                                    pt_[kc // 2][:, kc % 2, :],
                                    start=(kc == 0), stop=(kc == NKT - 1),
                                )
                    pv_norm(poa, 2 * prv, qcv)
                    pv_norm(pob, 2 * prv + 1, qcv)

                pts = {}
                squeue = deque(
                    (pr, qc) for qc in range(NQC) for pr in range(NH // 2)
                )
                pvqueue = deque()

                def emit_s_one():
                    if squeue:
                        pr, qc = squeue.popleft()
                        s_block(pr, qc, pts)
                        pvqueue.append((pr, qc))

                # Q projection: pass 1, then two score blocks (pr 0-1 at
                # qc 0 need only c-chunks 0-1), then pass 2, then the third
                # block — exps start right after Q pass 1 while the c3 pass
                # fills the PE during the exp drain.
                qk_proj_pass("q", qw_, qx, bqs_t if use_bias else None,
                             qpt, range(3))
                emit_s_one()
                emit_s_one()
                qk_proj_pass("q", qw_, qx, bqs_t if use_bias else None,
                             qpt, range(3, NJ))
                emit_s_one()

                # --- V projection (into [k, j] + ones column).  PSUM comes
                # from the po tag's banks (one [P, QW] bank per kc, pairs in
                # flight) so the V projection is NOT serialized behind the
                # score tiles' exp drain.
                vw, vx = load_wx(wv_d, vt_d, "v")
                nc.vector.tensor_copy(vext[:, :, :, HD:HD + 1], vof[:])
                for kcp in range(NKT // 2):
                    vg = {}
                    for i in range(2):
                        kc = 2 * kcp + i
                        vg[kc] = psum.tile([P, QW], F32, name="psv",
                                           tag="po", bufs=2)
                        if use_bias:
                            nc.tensor.matmul(
                                vg[kc][:],
                                ones_row[0:1, 0:P], bvs_t[:],
                                start=True, stop=False,
                                skip_group_check=True,
                            )
                    for dc in range(ND):
                        for i in range(2):
                            kc = 2 * kcp + i
                            nc.tensor.matmul(
                                vg[kc][:],
                                vx[dc // 2][:, dc % 2, kc * P:(kc + 1) * P],
                                vw[dc // 4][:, dc % 4, :],
                                start=(dc == 0 and not use_bias),
                                stop=(dc == ND - 1),
                                skip_group_check=True,
                            )
                    for i in range(2):
                        kc = 2 * kcp + i
                        nc.vector.tensor_copy(
                            vext[:, kc, :, 0:HD],
                            vg[kc][:].rearrange("p (h d) -> p h d", h=NH),
                        )

                wo_t = pwo.tile([P, NJ, OUT], DT, name="wo_t", tag="wo")
                nc.sync.dma_start(
                    wo_t[:], wo_d[:].rearrange("(a p) n -> p a n", p=P)
                )

                # --- Output projection per query-row chunk; jc outer so
                # each ot stationary slice serves both moving halves.
                def emit_final_qm(qm):
                    ps = psum.tile([P, NQC, QW], F32, name="psf", tag="ps")
                    for jc in range(NJ):
                        for oc in range(NQC):
                            nc.tensor.matmul(
                                ps[:, oc, :],
                                ot[:, jc, qm * P:(qm + 1) * P],
                                wo_t[:, jc, oc * QW:(oc + 1) * QW],
                                start=(jc == 0), stop=(jc == NJ - 1),
                                skip_group_check=True,
                            )
                    oc_t = pout.tile([P, OUT], OUT_DT, name="oct",
                                     tag="outc")
                    nc.vector.tensor_copy(
                        oc_t[:], ps[:].rearrange("p a q -> p (a q)")
                    )
                    nc.sync.dma_start(out_d[qm * P:(qm + 1) * P, :], oc_t[:])

                def emit_final(qhalf):
                    for qm in range(4 * qhalf, 4 * qhalf + 4):
                        emit_final_qm(qm)

                # --- drain remaining score blocks; P@V consumed FIFO so the
                # early blocks' P^T tiles free promptly; the qc=0 half of
                # the output projection interleaves with the qc=1 blocks.
                done_pv = 0
                finq = deque()
                while squeue:
                    cur = squeue.popleft()
                    fused_block(cur, pvqueue.popleft(), pts)
                    pvqueue.append(cur)
                    done_pv += 1
                    if done_pv == NH // 2:
                        finq.extend(range(4))  # qc0 output rows ready
                    if finq:
                        emit_final_qm(finq.popleft())
                    cur = None
                while finq:
                    emit_final_qm(finq.popleft())
                while pvqueue:
                    pr, qc = pvqueue.popleft()
                    pv_block(2 * pr, qc, pts.pop((2 * pr, qc)))
                    pv_block(2 * pr + 1, qc, pts.pop((2 * pr + 1, qc)))
                emit_final(1)

            if reps == 1:
                _emit_rep()
            else:
                with tc.For_i(0, reps, 1):
                    _emit_rep()

    nc.compile()
    return nc


_NC_CACHE = {}


def _get_nc(use_mask: bool, use_bias: bool = False, reps: int = 1):
    key = (use_mask, use_bias, reps)
    if key not in _NC_CACHE:
        _NC_CACHE[key] = _build(use_mask, use_bias, reps)
    return _NC_CACHE[key]


def _group_weights(Wq, bq, Wk, bk, Wv, bv, Wo, g):
    """Per-head-group weight slices in per-core layout j = head*64 + d."""
    scale = float(NUM_HEADS * HD) ** -0.5
    cols = np.array(
        [d * NUM_HEADS + (NH * g + hl) for hl in range(NH) for d in range(HD)]
    )
    return {
        "wq": np.ascontiguousarray((Wq[:, cols] * scale).astype(NPDT)),
        "bqs": np.ascontiguousarray((bq[cols] * scale)[None, :].astype(NPDT)),
        "wk": np.ascontiguousarray(Wk[:, cols].astype(NPDT)),
        "bks": np.ascontiguousarray(bk[cols][None, :].astype(NPDT)),
        "wv": np.ascontiguousarray(Wv[:, cols].astype(NPDT)),
        "bvs": np.ascontiguousarray(bv[cols][None, :].astype(NPDT)),
        "wo": np.ascontiguousarray(Wo[cols, :].astype(NPDT)),
    }


def make_in_maps(q, k, v, attn_mask, Wq, bq, Wk, bk, Wv, bv, Wo, bo):
    """Shard the full inputs into 8 per-core input maps."""
    use_mask = not bool(np.all(np.asarray(attn_mask) == 1.0))
    use_bias = bool(
        np.any(np.asarray(bq)) or np.any(np.asarray(bk)) or np.any(np.asarray(bv))
    )
    gw = [_group_weights(Wq, bq, Wk, bk, Wv, bv, Wo, g) for g in range(2)]
    xt = [
        {
            "qt": np.ascontiguousarray(np.asarray(q[b]).T.astype(NPDT)),
            "kt": np.ascontiguousarray(np.asarray(k[b]).T.astype(NPDT)),
            "vt": np.ascontiguousarray(np.asarray(v[b]).T.astype(NPDT)),
        }
        for b in range(B)
    ]
    mb = None
    if use_mask:
        mb = np.ascontiguousarray(
            (MASK_BIAS * (1.0 - np.asarray(attn_mask))).T.astype(NPDT)
        )
    in_maps = []
    for c in range(8):
        b, g = divmod(c, 2)
        m = dict(xt[b])
        m.update(gw[g])
        if not use_bias:
            for nm in ("bqs", "bks", "bvs"):
                m.pop(nm, None)
        if use_mask:
            m["mb"] = mb
        in_maps.append(m)
    return in_maps, use_mask, use_bias


def kernel(q, k, v, attn_mask, Wq, bq, Wk, bk, Wv, bv, Wo, bo):
    in_maps, use_mask, use_bias = make_in_maps(
        q, k, v, attn_mask, Wq, bq, Wk, bk, Wv, bv, Wo, bo
    )
    nc = _get_nc(use_mask, use_bias)
    res = run_bass_kernel_spmd(nc, in_maps, list(range(8)))
    out = np.empty((B, N, OUT), np.float32)
    bo = np.asarray(bo, np.float32)
    for b in range(B):
        out[b] = (
            res.results[2 * b]["out"].astype(np.float32)
            + res.results[2 * b + 1]["out"].astype(np.float32)
            + bo
        )
    return out



# revision 66
# speedup vs baseline: 1.3199x; 1.3199x over previous
"""Multi-head attention block (QKV projections + softmax attention + output
projection) for TRN2, distributed over 8 NeuronCores; fp16 data path with
fp32 PSUM accumulation, early-exp schedule.

Sharding: core c handles batch b = c // 2 and head group g = c % 2 (8 of the
16 heads).  Host sums the two partial outputs per batch and adds bo.

Schedule: kt+wk / qt+wq interleave on the SP DMA queue; K is projected
(2+2 chunk passes), then Q chunks 0-2, then TWO score blocks, the last Q
chunk, and a third score block, so the scalar engine's exp stream (the
serial floor, ~83us of ACTIVATE) starts right after Q pass 1.  The drain
consumes P@V tasks FIFO, fusing each remaining score block with the oldest
pending P@V pair at kc-pair granularity.

PSUM split (v3): 2 two-bank score/projection tiles + FOUR one-bank P@V
accumulators.  With only two po banks, the next block's P@V LDWEIGHTS
head-of-line-blocked the whole PE queue for ~5-7us per drain block
waiting on the pv_norm chain (a [1,512] single-partition DVE reciprocal
is ~4us), re-throttling the PE's HAM clock gate to 1.2 GHz for the
entire drain; four banks give the normalize chain two blocks of slack
and keep the PE at 2.4 GHz (measured 213.7us -> ~175us).

Score PSUM granularity (v4): one 2-bank score tile per kc holding BOTH
heads of the pair ([P, head, QW]; the pair is row-tiled in the PE), one
exp per kc over the pair.  With per-(head, kc-pair) tiles, a fused
block's first scores + an output-projection accumulator needed 3 tiles
from the 2-slot ps ring, so at every drain-block boundary the next
block's scores (and the ACT exp stream, the serial floor) serialized
behind a final's PSUM drain for ~1-3us; per-kc tiles keep one slot free
(measured ~175us -> ~167us).  pt tiles must be allocated UPFRONT per
block (allocating inside the kc loop measures +15us).

DMA queues (v5): kt/wv/wo streams on SP, qt on the (front-idle) ACT
queue, output writes on GpSimd — SP stays input-only, so in the reps
loop the next rep's input stream issues during this rep's drain instead
of after its last output DMA.

Final copies on ACT (v6/v7): ALL output-projection PSUM->SBUF copies
run on the scalar engine, not the DVE.  On the DVE queue a mid-drain
copy sits ~10us behind the pv_norm reciprocal chains, holding psf's
ps-ring slot and starving the score stream of slack at drain-block
boundaries; on ACT it costs the exp stream ~1.2us but frees the slot
promptly (measured ~167us -> 164.3us, with runs as low as 145us).
Do NOT put the tail RECIPROCALS on ACT via exp(-ln d): measured +20us
(ACT table churn between Ln and Exp); same for early score blocks on
the po ring (+18us, fences the V projection), interleaving tail finals
between pv pairs (neutral), and ppt bufs=36/40 (neutral).
Output projection matmuls run jc-outer so each stationary ot slice is
loaded once for both 512-column moving halves.
"""

from collections import deque
from contextlib import ExitStack

import numpy as np

import concourse.bass as bass
import concourse.tile as tile
from concourse import bacc, mybir
from concourse.bass_utils import run_bass_kernel_spmd

NUM_HEADS = 16
B = 4
N = 1024          # sequence length (n_q == n_k)
D = 1024          # model dim
OUT = 1024        # output dim
HD = 64           # head dim
NH = 8            # heads per core (16 heads / 2 groups)
J = NH * HD       # per-core all-head dim = 512
P = 128           # SBUF partitions
ND = D // P       # 8 contraction chunks over D
NKT = N // P      # 8 key-token chunks
NJ = J // P       # 4 chunks over j
QW = 512          # matmul moving width / PSUM bank width (fp32)
NQC = N // QW     # 2 query-column halves

F32 = mybir.dt.float32

DT = mybir.dt.float16
NPDT = np.float16
MASK_BIAS = -30000.0


def _build(use_mask: bool, use_bias: bool, reps: int = 1):
    nc = bacc.Bacc(None, target_bir_lowering=False)

    def din(nm, shape, dt=DT):
        return nc.declare_dram_parameter(nm, shape, dt, isOutput=False)

    qt_d, kt_d, vt_d = din("qt", [D, N]), din("kt", [D, N]), din("vt", [D, N])
    wq_d, wk_d, wv_d = din("wq", [D, J]), din("wk", [D, J]), din("wv", [D, J])
    wo_d = din("wo", [J, OUT])
    if use_bias:
        bqs_d, bks_d, bvs_d = (
            din("bqs", [1, J]), din("bks", [1, J]), din("bvs", [1, J])
        )
    mb_d = din("mb", [N, N], DT) if use_mask else None
    OUT_DT = DT
    out_d = nc.declare_dram_parameter("out", [N, OUT], OUT_DT, isOutput=True)

    AF = mybir.ActivationFunctionType
    big_bufs = 16

    with tile.TileContext(nc) as tc:
        with ExitStack() as ctx:
            # streaming slots for input chunks.
            big = ctx.enter_context(tc.tile_pool(name="big", bufs=big_bufs))
            # exp'd P^T tiles (256 KB fp16): own ring, sized for three
            # score blocks in flight plus margin.
            ppt = ctx.enter_context(tc.tile_pool(name="ppt", bufs=32))
            # NOTE: reciprocal_approx_fast (custom DVE op) measures ~5x
            # faster than reciprocal and passes in isolation, but corrupts
            # results inside this kernel on HW (sim passes) — do not use.
            pacts = ctx.enter_context(tc.tile_pool(name="acts", bufs=1))
            pwo = ctx.enter_context(tc.tile_pool(name="pwo", bufs=2))
            pout = ctx.enter_context(tc.tile_pool(name="outc", bufs=2))
            psml = ctx.enter_context(tc.tile_pool(name="small", bufs=1))
            # psum: 2 x 2-bank score/proj tiles + 4 x 1-bank P@V tiles.
            # Four po banks let two blocks' P@V results await pv_norm without
            # head-of-line-blocking the PE queue on the normalize chain.
            psum = ctx.enter_context(tc.tile_pool(name="ps", bufs=2, space="PSUM"))
            if use_mask:
                pmask = ctx.enter_context(tc.tile_pool(name="pmask", bufs=1))

            # Small persistent tiles.
            onesf = psml.tile([1, QW], F32, name="onesf", tag="onesf")
            nc.vector.memset(onesf[:], 1.0)
            if use_bias:
                ones_row = psml.tile([1, QW], DT, name="ones_row",
                                     tag="ones_row")
                nc.vector.tensor_copy(ones_row[:], onesf[:])
                bqs_t = psml.tile([1, J], DT, name="bqs_t", tag="bqs")
                bks_t = psml.tile([1, J], DT, name="bks_t", tag="bks")
                bvs_t = psml.tile([1, J], DT, name="bvs_t", tag="bvs")
                nc.sync.dma_start(bqs_t[:], bqs_d[:])
                nc.sync.dma_start(bks_t[:], bks_d[:])
                nc.sync.dma_start(bvs_t[:], bvs_d[:])
            vof = psml.tile([P, NKT, NH, 1], DT, name="vof", tag="vof")
            nc.vector.memset(vof[:], 1.0)

            # Warm the ACT exp table while DMAs run.
            warm = psml.tile([1, QW], F32, name="warm", tag="rc", bufs=2)
            nc.scalar.activation(warm[:], onesf[:], AF.Exp)

            if use_mask:
                mb_t = pmask.tile([P, NKT, N], DT, name="mb_t", tag="mask")
                nc.sync.dma_start(
                    mb_t[:], mb_d[:].rearrange("(a p) n -> p a n", p=P)
                )

            def _emit_rep():
                # --- input chunks, first-use order, single SP queue.
                # w: 2 tiles x [P, 4, J]; x: 4 tiles x [P, 2, N].
                def alloc_wx(nm):
                    w = [big.tile([P, 4, J], DT, name=f"w{nm}{i}", tag="big")
                         for i in range(2)]
                    x = [big.tile([P, 2, N], DT, name=f"x{nm}{i}", tag="big")
                         for i in range(4)]
                    return w, x

                def dma_wx(eng, wd, xd, w, x, order):
                    for lst, i in order:
                        if lst is w:
                            eng.dma_start(
                                w[i][:],
                                wd[i * (4 * P):(i + 1) * (4 * P), :].rearrange(
                                    "(a p) j -> p a j", p=P
                                ),
                            )
                        else:
                            eng.dma_start(
                                x[i][:],
                                xd[i * (2 * P):(i + 1) * (2 * P), :].rearrange(
                                    "(a p) n -> p a n", p=P
                                ),
                            )

                def load_wx(wd, xd, nm):
                    w, x = alloc_wx(nm)
                    dma_wx(nc.sync, wd, xd, w, x,
                           [(w, 0), (x, 0), (x, 1), (w, 1), (x, 2), (x, 3)])
                    return w, x

                qpt = pacts.tile([P, NJ, N], DT, name="qpt", tag="qpt")
                kpt = pacts.tile([P, NJ, N], DT, name="kpt", tag="kpt")
                vext = pacts.tile([P, NKT, NH, HD + 1], DT, name="vext",
                                  tag="vext")
                ot = pacts.tile([P, NJ, N], DT, name="ot", tag="ot")

                # --- Q/K projections over PSUM groups of 3 + 1 c-chunks.
                # po_banks: accumulate in po-tag banks instead of the ps
                # ring.  Used for the FIRST K pass so that, across reps,
                # it fences on the previous rep's tail P@V (frees ~12us
                # before rep end) rather than on the last output
                # projection's psf (frees at rep end) — the next rep's
                # first 32 matmuls overlap the previous rep's tail.
                def qk_proj_pass(nm, w, x, bias_t, dst, cs, po_banks=False):
                    if po_banks:
                        grp = {
                            c: [psum.tile([P, QW], F32, name="psp",
                                          tag="po", bufs=4)
                                for _ in range(NQC)]
                            for c in cs
                        }
                        tgt = lambda c, qc: grp[c][qc][:]
                    else:
                        grp = {
                            c: psum.tile([P, NQC, QW], F32, name="psp",
                                         tag="ps")
                            for c in cs
                        }
                        tgt = lambda c, qc: grp[c][:, qc, :]
                    if use_bias:
                        for c in cs:
                            for qc in range(NQC):
                                nc.tensor.matmul(
                                    tgt(c, qc),
                                    bias_t[0:1, c * P:(c + 1) * P],
                                    ones_row[:], start=True, stop=False,
                                    skip_group_check=True,
                                )
                    for dc in range(ND):
                        for c in cs:
                            for qc in range(NQC):
                                nc.tensor.matmul(
                                    tgt(c, qc),
                                    w[dc // 4][:, dc % 4,
                                               c * P:(c + 1) * P],
                                    x[dc // 2][:, dc % 2,
                                               qc * QW:(qc + 1) * QW],
                                    start=(dc == 0 and not use_bias),
                                    stop=(dc == ND - 1),
                                    skip_group_check=True,
                                )
                    for c in cs:
                        if po_banks:
                            for qc in range(NQC):
                                nc.vector.tensor_copy(
                                    dst[:, c, qc * QW:(qc + 1) * QW],
                                    grp[c][qc][:],
                                )
                        else:
                            nc.vector.tensor_copy(
                                dst[:, c, :],
                                grp[c][:].rearrange("p a q -> p (a q)"),
                            )

                # kt stream on the SP queue, qt stream on the (front-idle)
                # ACT queue: the two streams issue concurrently, and in the
                # reps loop the next rep's inputs no longer queue behind
                # this rep's output DMAs (which ride the GpSimd queue).
                kw, kx = alloc_wx("k")
                qw_, qx = alloc_wx("q")
                dma_wx(nc.sync, wk_d, kt_d, kw, kx,
                       [(kw, 0), (kx, 0), (kx, 1), (kw, 1), (kx, 2),
                        (kx, 3)])
                dma_wx(nc.scalar, wq_d, qt_d, qw_, qx,
                       [(qw_, 0), (qx, 0), (qx, 1), (qw_, 1), (qx, 2),
                        (qx, 3)])

                # --- Attention blocks; head pairs row-tiled in the PE.
                # Score PSUM is one 2-bank tile per kc holding BOTH heads
                # ([P, head, QW]), so the ps ring (2 slots) always has one
                # slot free for the output projection's accumulator while a
                # score tile is in flight — the next block's scores no
                # longer serialize behind a final's PSUM drain.
                def s_kc(pr, qc, kc, tiles, po_banks=False):
                    # po_banks: score PSUM from the (front-idle) po ring as
                    # two 1-bank tiles so early blocks don't contend with
                    # the projections' ps ring (costs one extra ACTIVATE's
                    # fixed overhead per kc).
                    if po_banks:
                        pss = [psum.tile([P, QW], F32, name="pss", tag="po",
                                         bufs=4) for _ in range(2)]
                        dst = lambda i: pss[i][:]
                    else:
                        psb = psum.tile([P, 2, QW], F32, name="pss",
                                        tag="ps")
                        dst = lambda i: psb[:, i, :]
                    for h in (2 * pr, 2 * pr + 1):
                        off = HD * (h & 1)
                        nc.tensor.matmul(
                            dst(h & 1),
                            kpt[off:off + HD, pr, kc * P:(kc + 1) * P],
                            qpt[off:off + HD, pr, qc * QW:(qc + 1) * QW],
                            start=True, stop=True,
                            skip_group_check=True,
                        )
                    if use_mask:
                        for i in range(2):
                            nc.vector.tensor_add(
                                dst(i), dst(i),
                                mb_t[:, kc, qc * QW:(qc + 1) * QW],
                            )
                    if po_banks:
                        for i in range(2):
                            nc.scalar.activation(tiles[kc][:, i, :],
                                                 pss[i][:], AF.Exp)
                    else:
                        nc.scalar.activation(tiles[kc][:], psb[:], AF.Exp)

                def alloc_pts(pts, pr, qc):
                    tiles = [
                        ppt.tile([P, 2, QW], DT, name="pt", tag="pt")
                        for _ in range(NKT)
                    ]
                    pts[(pr, qc)] = tiles
                    return tiles

                def s_block(pr, qc, pts, po_banks=False):
                    tiles = alloc_pts(pts, pr, qc)
                    for kc in range(NKT):
                        s_kc(pr, qc, kc, tiles, po_banks=po_banks)

                def pv_norm(po_, hv, qcv, act_recip=False):
                    rc = psml.tile([1, QW], F32, name="rc", tag="rc", bufs=2)
                    if act_recip:
                        # 1/d = exp(-ln d) on the scalar engine: used for
                        # half the tail norms so the two reciprocal chains
                        # run on ACT and DVE in parallel (the exp stream is
                        # done by then; a [1,512] DVE reciprocal is ~4us).
                        rl = psml.tile([1, QW], F32, name="rl", tag="rl",
                                       bufs=2)
                        nc.scalar.activation(rl[:], po_[HD:HD + 1, :], AF.Ln)
                        nc.scalar.activation(rc[:], rl[:], AF.Exp, scale=-1.0)
                    else:
                        nc.vector.reciprocal(rc[:], po_[HD:HD + 1, :])
                    rb = psml.tile([HD, QW], F32, name="rb", tag="rb", bufs=2)
                    nc.gpsimd.partition_broadcast(rb[:], rc[:])
                    off = HD * (hv & 1)
                    nc.vector.tensor_mul(
                        ot[off:off + HD, hv // 2, qcv * QW:(qcv + 1) * QW],
                        po_[0:HD, :], rb[:],
                    )

                def pv_block(h, qc, tiles):
                    po = psum.tile([P, QW], F32, name="po", tag="po",
                                   bufs=4)
                    for kc in range(NKT):
                        nc.tensor.matmul(
                            po[0:HD + 1, :], vext[:, kc, h, :],
                            tiles[kc][:, h & 1, :],
                            start=(kc == 0), stop=(kc == NKT - 1),
                        )
                    pv_norm(po, h, qc)

                def fused_block(s_task, pv_task, pts):
                    """Score block for s_task with the P@V pair for pv_task
                    interleaved at kc steps."""
                    pr, qc = s_task
                    prv, qcv = pv_task
                    tiles_v = pts.pop((prv, qcv))
                    poa = psum.tile([P, QW], F32, name="po", tag="po",
                                    bufs=4)
                    pob = psum.tile([P, QW], F32, name="po", tag="po",
                                    bufs=4)
                    tiles = alloc_pts(pts, pr, qc)
                    for kc in range(NKT):
                        s_kc(pr, qc, kc, tiles)
                        for po_, hv in ((poa, 2 * prv), (pob, 2 * prv + 1)):
                            nc.tensor.matmul(
                                po_[0:HD + 1, :], vext[:, kc, hv, :],
                                tiles_v[kc][:, hv & 1, :],
                                start=(kc == 0), stop=(kc == NKT - 1),
                            )
                    pv_norm(poa, 2 * prv, qcv)
                    pv_norm(pob, 2 * prv + 1, qcv)

                pts = {}
                squeue = deque(
                    (pr, qc) for qc in range(NQC) for pr in range(NH // 2)
                )
                pvqueue = deque()

                def emit_s_one(po_banks=False):
                    if squeue:
                        pr, qc = squeue.popleft()
                        s_block(pr, qc, pts, po_banks=po_banks)
                        pvqueue.append((pr, qc))

                # K fully, then Q chunks 0-2, then two score blocks, then
                # the last Q chunk, then the third block.  (Interleaving
                # K/Q passes at chunk granularity measures slower; so do
                # early score blocks on the po ring — they fence the V
                # projection's accumulators behind the exp stream, +18us.)
                bk = bks_t if use_bias else None
                bq = bqs_t if use_bias else None
                qk_proj_pass("k", kw, kx, bk, kpt, range(2))
                qk_proj_pass("k", kw, kx, bk, kpt, range(2, NJ))
                qk_proj_pass("q", qw_, qx, bq, qpt, range(3))
                emit_s_one()
                emit_s_one()
                qk_proj_pass("q", qw_, qx, bq, qpt, range(3, NJ))
                emit_s_one()

                # --- V projection (into [k, j] + ones column).  PSUM comes
                # from the po tag's banks (one [P, QW] bank per kc, pairs in
                # flight) so the V projection is NOT serialized behind the
                # score tiles' exp drain.
                vw, vx = load_wx(wv_d, vt_d, "v")
                nc.vector.tensor_copy(vext[:, :, :, HD:HD + 1], vof[:])
                for kcp in range(NKT // 2):
                    vg = {}
                    for i in range(2):
                        kc = 2 * kcp + i
                        vg[kc] = psum.tile([P, QW], F32, name="psv",
                                           tag="po", bufs=4)
                        if use_bias:
                            nc.tensor.matmul(
                                vg[kc][:],
                                ones_row[0:1, 0:P], bvs_t[:],
                                start=True, stop=False,
                                skip_group_check=True,
                            )
                    for dc in range(ND):
                        for i in range(2):
                            kc = 2 * kcp + i
                            nc.tensor.matmul(
                                vg[kc][:],
                                vx[dc // 2][:, dc % 2, kc * P:(kc + 1) * P],
                                vw[dc // 4][:, dc % 4, :],
                                start=(dc == 0 and not use_bias),
                                stop=(dc == ND - 1),
                                skip_group_check=True,
                            )
                    for i in range(2):
                        kc = 2 * kcp + i
                        nc.vector.tensor_copy(
                            vext[:, kc, :, 0:HD],
                            vg[kc][:].rearrange("p (h d) -> p h d", h=NH),
                        )

                wo_t = pwo.tile([P, NJ, OUT], DT, name="wo_t", tag="wo")
                nc.sync.dma_start(
                    wo_t[:], wo_d[:].rearrange("(a p) n -> p a n", p=P)
                )

                # --- Output projection per query-row chunk; jc outer so
                # each ot stationary slice serves both moving halves.
                # Tail finals (qc1, after the exp stream has drained) copy
                # PSUM->SBUF on the idle scalar engine instead of the DVE,
                # which is busy with the tail pv_norm reciprocal chains.
                def emit_final_qm(qm, tail=False):
                    psf = psum.tile([P, NQC, QW], F32, name="psf",
                                    tag="ps")
                    for jc in range(NJ):
                        for oc in range(NQC):
                            nc.tensor.matmul(
                                psf[:, oc, :],
                                ot[:, jc, qm * P:(qm + 1) * P],
                                wo_t[:, jc, oc * QW:(oc + 1) * QW],
                                start=(jc == 0), stop=(jc == NJ - 1),
                                skip_group_check=True,
                            )
                    oc_t = pout.tile([P, OUT], OUT_DT, name="oct",
                                     tag="outc")
                    src = psf[:].rearrange("p a q -> p (a q)")
                    # ALL finals copy on ACT: mid-drain, a DVE copy sits
                    # ~10us behind the pv_norm reciprocal chains holding
                    # psf's ps-ring slot (GpSimd can't read PSUM); on ACT it
                    # runs promptly, costing the exp stream ~1.2us but
                    # restoring the score ring's slack at block boundaries.
                    nc.scalar.copy(oc_t[:], src)
                    # GpSimd queue: keeps the SP queue input-only, so the
                    # next rep's kt stream is not fenced behind this rep's
                    # output writes.
                    nc.gpsimd.dma_start(out_d[qm * P:(qm + 1) * P, :],
                                        oc_t[:])

                def emit_final(qhalf):
                    for qm in range(4 * qhalf, 4 * qhalf + 4):
                        emit_final_qm(qm, tail=True)

                # --- drain remaining score blocks; P@V consumed FIFO so the
                # early blocks' P^T tiles free promptly; the qc=0 half of
                # the output projection interleaves with the qc=1 blocks.
                done_pv = 0
                finq = deque()
                while squeue:
                    cur = squeue.popleft()
                    fused_block(cur, pvqueue.popleft(), pts)
                    pvqueue.append(cur)
                    done_pv += 1
                    if done_pv == NH // 2:
                        finq.extend(range(4))  # qc0 output rows ready
                    if finq:
                        emit_final_qm(finq.popleft())
                    cur = None
                # Interleave the remaining qc0 finals between the tail P@V
                # pairs: their matmuls keep the PE fed while the pairs'
                # normalize chains (the 4us DVE reciprocals) trail.
                while pvqueue:
                    pr, qc = pvqueue.popleft()
                    tiles = pts.pop((pr, qc))
                    pv_block(2 * pr, qc, tiles)
                    pv_block(2 * pr + 1, qc, tiles)
                    if finq:
                        emit_final_qm(finq.popleft())
                while finq:
                    emit_final_qm(finq.popleft())
                emit_final(1)

            if reps == 1:
                _emit_rep()
            else:
                with tc.For_i(0, reps, 1):
                    _emit_rep()

    nc.compile()
    return nc


_NC_CACHE = {}


def _get_nc(use_mask: bool, use_bias: bool = False, reps: int = 1):
    key = (use_mask, use_bias, reps)
    if key not in _NC_CACHE:
        _NC_CACHE[key] = _build(use_mask, use_bias, reps)
    return _NC_CACHE[key]


def _group_weights(Wq, bq, Wk, bk, Wv, bv, Wo, g):
    """Per-head-group weight slices in per-core layout j = head*64 + d."""
    scale = float(NUM_HEADS * HD) ** -0.5
    cols = np.array(
        [d * NUM_HEADS + (NH * g + hl) for hl in range(NH) for d in range(HD)]
    )
    return {
        "wq": np.ascontiguousarray((Wq[:, cols] * scale).astype(NPDT)),
        "bqs": np.ascontiguousarray((bq[cols] * scale)[None, :].astype(NPDT)),
        "wk": np.ascontiguousarray(Wk[:, cols].astype(NPDT)),
        "bks": np.ascontiguousarray(bk[cols][None, :].astype(NPDT)),
        "wv": np.ascontiguousarray(Wv[:, cols].astype(NPDT)),
        "bvs": np.ascontiguousarray(bv[cols][None, :].astype(NPDT)),
        "wo": np.ascontiguousarray(Wo[cols, :].astype(NPDT)),
    }


def make_in_maps(q, k, v, attn_mask, Wq, bq, Wk, bk, Wv, bv, Wo, bo):
    """Shard the full inputs into 8 per-core input maps."""
    use_mask = not bool(np.all(np.asarray(attn_mask) == 1.0))
    use_bias = bool(
        np.any(np.asarray(bq)) or np.any(np.asarray(bk)) or np.any(np.asarray(bv))
    )
    gw = [_group_weights(Wq, bq, Wk, bk, Wv, bv, Wo, g) for g in range(2)]
    xt = [
        {
            "qt": np.ascontiguousarray(np.asarray(q[b]).T.astype(NPDT)),
            "kt": np.ascontiguousarray(np.asarray(k[b]).T.astype(NPDT)),
            "vt": np.ascontiguousarray(np.asarray(v[b]).T.astype(NPDT)),
        }
        for b in range(B)
    ]
    mb = None
    if use_mask:
        mb = np.ascontiguousarray(
            (MASK_BIAS * (1.0 - np.asarray(attn_mask))).T.astype(NPDT)
        )
    in_maps = []
    for c in range(8):
        b, g = divmod(c, 2)
        m = dict(xt[b])
        m.update(gw[g])
        if not use_bias:
            for nm in ("bqs", "bks", "bvs"):
                m.pop(nm, None)
        if use_mask:
            m["mb"] = mb
        in_maps.append(m)
    return in_maps, use_mask, use_bias


def kernel(q, k, v, attn_mask, Wq, bq, Wk, bk, Wv, bv, Wo, bo):
    in_maps, use_mask, use_bias = make_in_maps(
        q, k, v, attn_mask, Wq, bq, Wk, bk, Wv, bv, Wo, bo
    )
    nc = _get_nc(use_mask, use_bias)
    res = run_bass_kernel_spmd(nc, in_maps, list(range(8)))
    out = np.empty((B, N, OUT), np.float32)
    bo = np.asarray(bo, np.float32)
    for b in range(B):
        out[b] = (
            res.results[2 * b]["out"].astype(np.float32)
            + res.results[2 * b + 1]["out"].astype(np.float32)
            + bo
        )
    return out

